# revision 34
# baseline (speedup 1.0000x reference)
"""AttentionWide (t=2048, e=512, h=8) on 8 TRN2 NeuronCores.

Tensor-parallel over heads: core i owns head i (columns i*512:(i+1)*512 of
Wk/Wq/Wv, rows i*512:(i+1)*512 of Wu).  Each core computes its head's
attention and the partial unifyheads product; chunked ReduceScatters sum the
partials across cores, each core returning row-shards of the final output.

Weight folding (host-side, exact algebra — the head dim equals emb here so
no information is lost):
    scores = q k^T = (y Wq)(x Wk)^T = y (Wq Wk^T) x^T
      ->  G  = Wk Wq^T   [e, e]   (host)
          g  = x G                 (device)
          scoresT[tk, tq] = g y^T  (device)
    out = attn @ v @ Wu = attn @ (x Wv Wu)
      ->  W2 = Wv Wu     [e, e]   (host)
          vW = x W2                (device)
This removes two of the four projection matmul groups.

Device compute in bf16 with fp32 PSUM accumulation, in "transposed"
layouts so no on-device transposes are needed (x/y transposed on host):
    gT = G^T @ xT            lhsT=G (natural),  rhs=xT       [e, t]
    vW = xT^T @ W2           lhsT=xT slices,    rhs=W2       [t, e]
    scoresT[tk,tq] = gT^T yT lhsT=gT slices,    rhs=yT
    expT = exp(scoresT * e^-0.5)  (softmax w/o max-subtraction; |scores|<~2)
    out[tq,:] = (expT^T @ vW) / (expT^T @ ones)  lhsT=expT slices (shared)
"""

import os
import numpy as np
import ml_dtypes

T, E, H = 2048, 512, 8
NCORES = 8
TB = 512          # matmul moving-operand block (free dim; one fp32 PSUM bank)
NE = E // 128     # 4  partition tiles of the emb dim
NT = T // 128     # 16 partition tiles of the seq dim
NB = T // TB      # 4  seq blocks
# ReduceScatter chunks (rows per chunk, sum = T).  The first two compute
# blocks share one chunk: with only two collectives ahead of the last one
# (both complete long before the final chunk's input is ready), the last —
# fully exposed — ReduceScatter can never queue up behind the CC chain on
# a skewed core.
CHUNKS = [1024, 512, 512]
NCH = len(CHUNKS)
CHUNK_R0 = [sum(CHUNKS[:c]) for c in range(NCH)]
EP = E + 1        # vW columns + folded ones column (softmax denominator)
NA = 257          # first-half free dim of the split final matmul
NBC = EP - NA     # 256; its last column holds the denominator

_cache = {}
last_result = None


def _build_nc():
    from concourse import bacc, tile
    from concourse.bass import mybir

    bf16 = mybir.dt.bfloat16
    f16 = mybir.dt.float16
    f32 = mybir.dt.float32

    nc = bacc.Bacc(
        "TRN2", target_bir_lowering=False, debug=False, num_devices=NCORES
    )

    xT = nc.dram_tensor("xT", [E, T], bf16, kind="ExternalInput")
    yT = nc.dram_tensor("yT", [E, T], bf16, kind="ExternalInput")
    gw = nc.dram_tensor("gw", [E, E], bf16, kind="ExternalInput")   # Wk Wq^T
    w2 = nc.dram_tensor("w2", [E, E], bf16, kind="ExternalInput")   # Wv Wu
    # fp16 reduction payload: the partials are ~N(0, 0.1^2) so fp16 keeps
    # ~3 more mantissa bits than bf16 and halves the collective bytes.
    out_exts = [
        nc.dram_tensor(f"out{c}", [CHUNKS[c] // NCORES, E], f16, kind="ExternalOutput")
        for c in range(NCH)
    ]

    with tile.TileContext(nc) as tc:
        with (
            tc.tile_pool(name="persist", bufs=1) as persist,
            tc.tile_pool(name="work", bufs=4) as work,
            tc.tile_pool(name="expp", bufs=32) as expp,
            tc.tile_pool(name="psum", bufs=2, space="PSUM") as psum_pool,
            tc.tile_pool(name="dram", bufs=1, space="DRAM") as dram,
        ):
            def alloc_rows(prefix, n):
                return [
                    persist.tile(
                        [128, n], bf16, tag=f"{prefix}{j}", name=f"{prefix}{j}"
                    )
                    for j in range(NE)
                ]

            xT_sb = alloc_rows("xTs", T)
            yT_sb = alloc_rows("yTs", T)
            gw_sb = alloc_rows("gws", E)
            w2_sb = alloc_rows("w2s", E)

            # DMA order = need order: gw, xT col-chunk 0, w2, rest of xT,
            # then yT per column block.  Column-chunked so the first
            # projection matmuls can start after ~1MB has landed.
            # first-needed tiles split in half so they spread across more
            # DMA queues and the first matmul can start sooner
            for j in range(NE):
                for h in range(2):
                    nc.sync.dma_start(
                        gw_sb[j][:, h * 256 : (h + 1) * 256],
                        gw[j * 128 : (j + 1) * 128, h * 256 : (h + 1) * 256],
                    )
                for h in range(2):
                    nc.sync.dma_start(
                        xT_sb[j][:, h * 256 : (h + 1) * 256],
                        xT[j * 128 : (j + 1) * 128, h * 256 : (h + 1) * 256],
                    )
            # the gT phase consumes one xT column block per ~3.3us, racing the
            # DMA — quarter the transfers so every queue helps deliver the
            # next-needed block
            for tb in range(1, NB):
                for j in range(NE):
                    for h in range(4):
                        c0 = tb * TB + h * 128
                        nc.sync.dma_start(
                            xT_sb[j][:, c0 : c0 + 128],
                            xT[j * 128 : (j + 1) * 128, c0 : c0 + 128],
                        )
            for j in range(NE):
                nc.sync.dma_start(w2_sb[j][:], w2[j * 128 : (j + 1) * 128, :])
            for tb in range(NB):
                for j in range(NE):
                    nc.sync.dma_start(
                        yT_sb[j][:, tb * TB : (tb + 1) * TB],
                        yT[j * 128 : (j + 1) * 128, tb * TB : (tb + 1) * TB],
                    )

            gT_sb = alloc_rows("gTs", T)
            # vW plus a folded ones column: col E is 1.0, so the final
            # matmul's second half also produces the softmax denominator.
            vW_sb = [
                persist.tile([128, EP], bf16, tag=f"vWs{t}", name=f"vWs{t}")
                for t in range(NT)
            ]

            zbias = persist.tile([128, 1], f32, tag="zbias", name="zbias")
            nc.vector.memset(zbias[:], 0.0)

            # Warm up the PE clock (HAM) during the initial DMA wait: dummy
            # matmuls on a zeroed tile keep TensorE busy so the ~3.4us
            # cold-clock ramp overlaps the input load instead of the first
            # real matmuls.
            if os.environ.get("KERNEL_WARMUP", "1") == "1":
                warm = persist.tile([128, TB], bf16, tag="warm", name="warm")
                nc.vector.memset(warm[:], 0.0)
                for w in range(12):
                    pw = psum_pool.tile(
                        [128, TB], f32, tag="mm", bufs=4, name="pw"
                    )
                    nc.tensor.matmul(
                        pw[:], warm[:, 0:128], warm[:], start=True, stop=True
                    )

            # gT[m][:, tk] = sum_j G[j][:, m-slice].T @ xT[j][:, tk-block]
            # tb-major so the first 16 matmuls only need xT's first column
            # block (the rest still stream in)
            for tb in range(NB):
                for m in range(NE):
                    ps = psum_pool.tile(
                        [128, TB], f32, tag="mm", bufs=4, name="ps_g"
                    )
                    for j in range(NE):
                        nc.tensor.matmul(
                            ps[:],
                            gw_sb[j][:, m * 128 : (m + 1) * 128],
                            xT_sb[j][:, tb * TB : (tb + 1) * TB],
                            start=(j == 0),
                            stop=(j == NE - 1),
                        )
                    nc.vector.tensor_copy(gT_sb[m][:, tb * TB : (tb + 1) * TB], ps[:])

            # vW[t, :] = x @ W2   (natural [t, e] layout), ones in col E
            for t in range(NT):
                ps = psum_pool.tile([128, E], f32, tag="mm", bufs=4, name="ps_vw")
                for j in range(NE):
                    nc.tensor.matmul(
                        ps[:],
                        xT_sb[j][:, t * 128 : (t + 1) * 128],
                        w2_sb[j][:],
                        start=(j == 0),
                        stop=(j == NE - 1),
                    )
                nc.vector.memset(vW_sb[t][:, E:EP], 1.0)
                nc.vector.tensor_copy(vW_sb[t][:, 0:E], ps[:])

            SCALE = float(E) ** -0.5
            parts = [
                dram.tile([CHUNKS[c], E], f16, tag=f"part{c}", name=f"part{c}")
                for c in range(NCH)
            ]
            rs_outs = [
                dram.tile(
                    [CHUNKS[c] // NCORES, E], f16, tag=f"rso{c}", name=f"rso{c}"
                )
                for c in range(NCH)
            ]
            # (chunk, row-tile within chunk) for each global 128-row tile
            tile2chunk = [
                (c, r) for c, nr in enumerate(CHUNKS) for r in range(nr // 128)
            ]

            for b in range(NB):
                # scoresT[tk, tq-block b] then exp
                exp_tiles = []
                for tk in range(NT):
                    ps = psum_pool.tile(
                        [128, TB], f32, tag="mm", bufs=4, name="ps_sc"
                    )
                    for m in range(NE):
                        nc.tensor.matmul(
                            ps[:],
                            gT_sb[m][:, tk * 128 : (tk + 1) * 128],
                            yT_sb[m][:, b * TB : (b + 1) * TB],
                            start=(m == 0),
                            stop=(m == NE - 1),
                        )
                    et = expp.tile([128, TB], bf16, tag="expT", bufs=32, name="et")
                    nc.scalar.activation(
                        et[:],
                        ps[:],
                        mybir.ActivationFunctionType.Exp,
                        bias=zbias[:],
                        scale=SCALE,
                    )
                    exp_tiles.append(et)

                # out rows for this block: accumulate over tk, then normalize.
                # The EP=513 free dim is split 257+256 across two PSUM banks;
                # the last column of pb is the softmax denominator.
                for qi in range(TB // 128):
                    pa = psum_pool.tile([128, NA], f32, tag="acca", bufs=2, name="pa")
                    pb = psum_pool.tile([128, NBC], f32, tag="accb", bufs=2, name="pb")
                    for tk in range(NT):
                        lhs = exp_tiles[tk][:, qi * 128 : (qi + 1) * 128]
                        nc.tensor.matmul(
                            pa[:],
                            lhs,
                            vW_sb[tk][:, 0:NA],
                            start=(tk == 0),
                            stop=(tk == NT - 1),
                        )
                        nc.tensor.matmul(
                            pb[:],
                            lhs,
                            vW_sb[tk][:, NA:EP],
                            start=(tk == 0),
                            stop=(tk == NT - 1),
                        )
                    rec = work.tile([128, 1], f32, tag="rec", bufs=4, name="rec")
                    nc.vector.reciprocal(rec[:], pb[:, NBC - 1 : NBC])
                    ot = work.tile([128, E], f16, tag="ot", bufs=4, name="ot")
                    nc.vector.tensor_scalar_mul(ot[:, 0:NA], pa[:], rec[:])
                    nc.scalar.mul(ot[:, NA:E], pb[:, 0 : NBC - 1], rec[:])
                    ch, r = tile2chunk[b * (TB // 128) + qi]
                    nc.sync.dma_start(
                        parts[ch][r * 128 : (r + 1) * 128, :], ot[:]
                    )
                    if r == CHUNKS[ch] // 128 - 1:
                        nc.gpsimd.collective_compute(
                            "ReduceScatter",
                            mybir.AluOpType.add,
                            replica_groups=[list(range(NCORES))],
                            ins=[parts[ch][:]],
                            outs=[rs_outs[ch][:]],
                        )
                        nc.sync.dma_start(out_exts[ch][:], rs_outs[ch][:])

    nc.compile()
    return nc


def kernel(x, y, Wk, Wq, Wv, Wu, bu):
    global last_result
    from concourse.bass_utils import run_bass_kernel_spmd

    if "nc" not in _cache:
        _cache["nc"] = _build_nc()
    nc = _cache["nc"]

    bf = ml_dtypes.bfloat16
    x = np.asarray(x, np.float32)
    y = np.asarray(y, np.float32)
    Wk = np.asarray(Wk, np.float32)
    Wq = np.asarray(Wq, np.float32)
    Wv = np.asarray(Wv, np.float32)
    Wu = np.asarray(Wu, np.float32)

    xT = np.ascontiguousarray(x.T).astype(bf)
    yT = np.ascontiguousarray(y.T).astype(bf)

    in_maps = []
    for i in range(NCORES):
        sl = slice(i * E, (i + 1) * E)
        G = Wk[:, sl] @ Wq[:, sl].T        # [e, e] fp32 on host
        W2 = Wv[:, sl] @ Wu[sl, :]         # [e, e] fp32 on host
        in_maps.append(
            {
                "xT": xT,
                "yT": yT,
                "gw": G.astype(bf),
                "w2": W2.astype(bf),
            }
        )

    trace = os.environ.get("KERNEL_TRACE", "0") == "1"
    res = run_bass_kernel_spmd(
        nc, in_maps, core_ids=list(range(NCORES)), trace=trace
    )
    last_result = res

    out_full = np.empty((T, E), np.float32)
    chunk_r0 = np.cumsum([0] + CHUNKS)[:-1]
    for i in range(NCORES):
        for c in range(NCH):
            nr = CHUNKS[c] // NCORES
            o = np.asarray(res.results[i][f"out{c}"]).astype(np.float32)
            r0 = chunk_r0[c] + i * nr
            out_full[r0 : r0 + nr] = o
    out_full = out_full + np.asarray(bu, np.float32)[None, :]
    return out_full[None]


# revision 35
# speedup vs baseline: 1.0461x; 1.0461x over previous
"""AttentionWide (t=2048, e=512, h=8) on 8 TRN2 NeuronCores.

Tensor-parallel over heads: core i owns head i (columns i*512:(i+1)*512 of
Wk/Wq/Wv, rows i*512:(i+1)*512 of Wu).  Each core computes its head's
attention and the partial unifyheads product; chunked ReduceScatters sum the
partials across cores, each core returning row-shards of the final output.

Weight folding (host-side, exact algebra — the head dim equals emb here so
no information is lost):
    scores = q k^T = (y Wq)(x Wk)^T = y (Wq Wk^T) x^T
      ->  G  = Wk Wq^T   [e, e]   (host)
          g  = x G                 (device)
          scoresT[tk, tq] = g y^T  (device)
    out = attn @ v @ Wu = attn @ (x Wv Wu)
      ->  W2 = Wv Wu     [e, e]   (host)
          vW = x W2                (device)
This removes two of the four projection matmul groups.

Device compute in bf16 with fp32 PSUM accumulation, in "transposed"
layouts so no on-device transposes are needed (x/y transposed on host):
    gT = G^T @ xT            lhsT=G (natural),  rhs=xT       [e, t]
    vW = xT^T @ W2           lhsT=xT slices,    rhs=W2       [t, e]
    scoresT[tk,tq] = gT^T yT lhsT=gT slices,    rhs=yT
    expT = exp(scoresT * e^-0.5)  (softmax w/o max-subtraction; |scores|<~2)
    out[tq,:] = (expT^T @ vW) / (expT^T @ ones)  lhsT=expT slices (shared)
"""

import os
import numpy as np
import ml_dtypes

T, E, H = 2048, 512, 8
NCORES = 8
TB = 512          # matmul moving-operand block (free dim; one fp32 PSUM bank)
NE = E // 128     # 4  partition tiles of the emb dim
NT = T // 128     # 16 partition tiles of the seq dim
NB = T // TB      # 4  seq blocks
# ReduceScatter chunks (rows per chunk, sum = T).  The first two compute
# blocks share one chunk: with only two collectives ahead of the last one
# (both complete long before the final chunk's input is ready), the last —
# fully exposed — ReduceScatter can never queue up behind the CC chain on
# a skewed core.
CHUNKS = [1024, 512, 512]
NCH = len(CHUNKS)
CHUNK_R0 = [sum(CHUNKS[:c]) for c in range(NCH)]
EP = E + 1        # vW columns + folded ones column (softmax denominator)
NA = 257          # first-half free dim of the split final matmul
NBC = EP - NA     # 256; its last column holds the denominator

_cache = {}
last_result = None


def _build_nc():
    from concourse import bacc, tile
    from concourse.bass import mybir

    bf16 = mybir.dt.bfloat16
    f16 = mybir.dt.float16
    f32 = mybir.dt.float32

    nc = bacc.Bacc(
        "TRN2", target_bir_lowering=False, debug=False, num_devices=NCORES
    )

    xT = nc.dram_tensor("xT", [E, T], bf16, kind="ExternalInput")
    yT = nc.dram_tensor("yT", [E, T], bf16, kind="ExternalInput")
    gw = nc.dram_tensor("gw", [E, E], bf16, kind="ExternalInput")   # Wk Wq^T
    w2 = nc.dram_tensor("w2", [E, E], bf16, kind="ExternalInput")   # Wv Wu
    # fp16 reduction payload: the partials are ~N(0, 0.1^2) so fp16 keeps
    # ~3 more mantissa bits than bf16 and halves the collective bytes.
    out_exts = [
        nc.dram_tensor(f"out{c}", [CHUNKS[c] // NCORES, E], f16, kind="ExternalOutput")
        for c in range(NCH)
    ]

    with tile.TileContext(nc) as tc:
        with (
            tc.tile_pool(name="persist", bufs=1) as persist,
            tc.tile_pool(name="work", bufs=4) as work,
            tc.tile_pool(name="expp", bufs=32) as expp,
            tc.tile_pool(name="psum", bufs=2, space="PSUM") as psum_pool,
            tc.tile_pool(name="dram", bufs=1, space="DRAM") as dram,
        ):
            def alloc_rows(prefix, n):
                return [
                    persist.tile(
                        [128, n], bf16, tag=f"{prefix}{j}", name=f"{prefix}{j}"
                    )
                    for j in range(NE)
                ]

            xT_sb = alloc_rows("xTs", T)
            yT_sb = alloc_rows("yTs", T)
            gw_sb = alloc_rows("gws", E)
            w2_sb = alloc_rows("w2s", E)

            # DMA order = need order: gw, xT col-chunk 0, w2, rest of xT,
            # then yT per column block.  Column-chunked so the first
            # projection matmuls can start after ~1MB has landed.
            # first-needed tiles split in half so they spread across more
            # DMA queues and the first matmul can start sooner
            for j in range(NE):
                for h in range(2):
                    nc.sync.dma_start(
                        gw_sb[j][:, h * 256 : (h + 1) * 256],
                        gw[j * 128 : (j + 1) * 128, h * 256 : (h + 1) * 256],
                    )
                for h in range(2):
                    nc.sync.dma_start(
                        xT_sb[j][:, h * 256 : (h + 1) * 256],
                        xT[j * 128 : (j + 1) * 128, h * 256 : (h + 1) * 256],
                    )
            for tb in range(1, NB):
                for j in range(NE):
                    nc.sync.dma_start(
                        xT_sb[j][:, tb * TB : (tb + 1) * TB],
                        xT[j * 128 : (j + 1) * 128, tb * TB : (tb + 1) * TB],
                    )
            for j in range(NE):
                nc.sync.dma_start(w2_sb[j][:], w2[j * 128 : (j + 1) * 128, :])
            for tb in range(NB):
                for j in range(NE):
                    nc.sync.dma_start(
                        yT_sb[j][:, tb * TB : (tb + 1) * TB],
                        yT[j * 128 : (j + 1) * 128, tb * TB : (tb + 1) * TB],
                    )

            gT_sb = alloc_rows("gTs", T)
            # vW plus a folded ones column: col E is 1.0, so the final
            # matmul's second half also produces the softmax denominator.
            vW_sb = [
                persist.tile([128, EP], bf16, tag=f"vWs{t}", name=f"vWs{t}")
                for t in range(NT)
            ]

            zbias = persist.tile([128, 1], f32, tag="zbias", name="zbias")
            nc.vector.memset(zbias[:], 0.0)

            # Warm up the PE clock (HAM) during the initial DMA wait: dummy
            # matmuls on a zeroed tile keep TensorE busy so the ~3.4us
            # cold-clock ramp overlaps the input load instead of the first
            # real matmuls.
            if os.environ.get("KERNEL_WARMUP", "1") == "1":
                warm = persist.tile([128, TB], bf16, tag="warm", name="warm")
                nc.vector.memset(warm[:], 0.0)
                for w in range(12):
                    pw = psum_pool.tile(
                        [128, TB], f32, tag="mm", bufs=4, name="pw"
                    )
                    nc.tensor.matmul(
                        pw[:], warm[:, 0:128], warm[:], start=True, stop=True
                    )

            # gT[m][:, tk] = sum_j G[j][:, m-slice].T @ xT[j][:, tk-block]
            # tb-major so the first 16 matmuls only need xT's first column
            # block (the rest still stream in)
            for tb in range(NB):
                for m in range(NE):
                    ps = psum_pool.tile(
                        [128, TB], f32, tag="mm", bufs=4, name="ps_g"
                    )
                    for j in range(NE):
                        nc.tensor.matmul(
                            ps[:],
                            gw_sb[j][:, m * 128 : (m + 1) * 128],
                            xT_sb[j][:, tb * TB : (tb + 1) * TB],
                            start=(j == 0),
                            stop=(j == NE - 1),
                        )
                    nc.vector.tensor_copy(gT_sb[m][:, tb * TB : (tb + 1) * TB], ps[:])

            # vW[t, :] = x @ W2   (natural [t, e] layout), ones in col E
            for t in range(NT):
                ps = psum_pool.tile([128, E], f32, tag="mm", bufs=4, name="ps_vw")
                for j in range(NE):
                    nc.tensor.matmul(
                        ps[:],
                        xT_sb[j][:, t * 128 : (t + 1) * 128],
                        w2_sb[j][:],
                        start=(j == 0),
                        stop=(j == NE - 1),
                    )
                nc.vector.memset(vW_sb[t][:, E:EP], 1.0)
                nc.vector.tensor_copy(vW_sb[t][:, 0:E], ps[:])

            SCALE = float(E) ** -0.5
            parts = [
                dram.tile([CHUNKS[c], E], f16, tag=f"part{c}", name=f"part{c}")
                for c in range(NCH)
            ]
            rs_outs = [
                dram.tile(
                    [CHUNKS[c] // NCORES, E], f16, tag=f"rso{c}", name=f"rso{c}"
                )
                for c in range(NCH)
            ]
            # (chunk, row-tile within chunk) for each global 128-row tile
            tile2chunk = [
                (c, r) for c, nr in enumerate(CHUNKS) for r in range(nr // 128)
            ]

            for b in range(NB):
                # scoresT[tk, tq-block b] then exp
                exp_tiles = []
                for tk in range(NT):
                    ps = psum_pool.tile(
                        [128, TB], f32, tag="mm", bufs=4, name="ps_sc"
                    )
                    for m in range(NE):
                        nc.tensor.matmul(
                            ps[:],
                            gT_sb[m][:, tk * 128 : (tk + 1) * 128],
                            yT_sb[m][:, b * TB : (b + 1) * TB],
                            start=(m == 0),
                            stop=(m == NE - 1),
                        )
                    et = expp.tile([128, TB], bf16, tag="expT", bufs=32, name="et")
                    nc.scalar.activation(
                        et[:],
                        ps[:],
                        mybir.ActivationFunctionType.Exp,
                        bias=zbias[:],
                        scale=SCALE,
                    )
                    exp_tiles.append(et)

                # out rows for this block: accumulate over tk, then normalize.
                # The EP=513 free dim is split 257+256 across two PSUM banks;
                # the last column of pb is the softmax denominator.
                for qi in range(TB // 128):
                    pa = psum_pool.tile([128, NA], f32, tag="acca", bufs=2, name="pa")
                    pb = psum_pool.tile([128, NBC], f32, tag="accb", bufs=2, name="pb")
                    for tk in range(NT):
                        lhs = exp_tiles[tk][:, qi * 128 : (qi + 1) * 128]
                        nc.tensor.matmul(
                            pa[:],
                            lhs,
                            vW_sb[tk][:, 0:NA],
                            start=(tk == 0),
                            stop=(tk == NT - 1),
                        )
                        nc.tensor.matmul(
                            pb[:],
                            lhs,
                            vW_sb[tk][:, NA:EP],
                            start=(tk == 0),
                            stop=(tk == NT - 1),
                        )
                    rec = work.tile([128, 1], f32, tag="rec", bufs=4, name="rec")
                    nc.vector.reciprocal(rec[:], pb[:, NBC - 1 : NBC])
                    ot = work.tile([128, E], f16, tag="ot", bufs=4, name="ot")
                    nc.vector.tensor_scalar_mul(ot[:, 0:NA], pa[:], rec[:])
                    nc.scalar.mul(ot[:, NA:E], pb[:, 0 : NBC - 1], rec[:])
                    ch, r = tile2chunk[b * (TB // 128) + qi]
                    nc.sync.dma_start(
                        parts[ch][r * 128 : (r + 1) * 128, :], ot[:]
                    )
                    if r == CHUNKS[ch] // 128 - 1:
                        nc.gpsimd.collective_compute(
                            "ReduceScatter",
                            mybir.AluOpType.add,
                            replica_groups=[list(range(NCORES))],
                            ins=[parts[ch][:]],
                            outs=[rs_outs[ch][:]],
                        )
                        nc.sync.dma_start(out_exts[ch][:], rs_outs[ch][:])

    nc.compile()
    return nc


def kernel(x, y, Wk, Wq, Wv, Wu, bu):
    global last_result
    from concourse.bass_utils import run_bass_kernel_spmd

    if "nc" not in _cache:
        _cache["nc"] = _build_nc()
    nc = _cache["nc"]

    bf = ml_dtypes.bfloat16
    x = np.asarray(x, np.float32)
    y = np.asarray(y, np.float32)
    Wk = np.asarray(Wk, np.float32)
    Wq = np.asarray(Wq, np.float32)
    Wv = np.asarray(Wv, np.float32)
    Wu = np.asarray(Wu, np.float32)

    xT = np.ascontiguousarray(x.T).astype(bf)
    yT = np.ascontiguousarray(y.T).astype(bf)

    in_maps = []
    for i in range(NCORES):
        sl = slice(i * E, (i + 1) * E)
        G = Wk[:, sl] @ Wq[:, sl].T        # [e, e] fp32 on host
        W2 = Wv[:, sl] @ Wu[sl, :]         # [e, e] fp32 on host
        in_maps.append(
            {
                "xT": xT,
                "yT": yT,
                "gw": G.astype(bf),
                "w2": W2.astype(bf),
            }
        )

    trace = os.environ.get("KERNEL_TRACE", "0") == "1"
    res = run_bass_kernel_spmd(
        nc, in_maps, core_ids=list(range(NCORES)), trace=trace
    )
    last_result = res

    out_full = np.empty((T, E), np.float32)
    chunk_r0 = np.cumsum([0] + CHUNKS)[:-1]
    for i in range(NCORES):
        for c in range(NCH):
            nr = CHUNKS[c] // NCORES
            o = np.asarray(res.results[i][f"out{c}"]).astype(np.float32)
            r0 = chunk_r0[c] + i * nr
            out_full[r0 : r0 + nr] = o
    out_full = out_full + np.asarray(bu, np.float32)[None, :]
    return out_full[None]


# revision 38
# speedup vs baseline: 1.0756x; 1.0283x over previous
"""AttentionWide (t=2048, e=512, h=8) on 8 TRN2 NeuronCores.

Tensor-parallel over heads: core i owns head i (columns i*512:(i+1)*512 of
Wk/Wq/Wv, rows i*512:(i+1)*512 of Wu).  Each core computes its head's
attention and the partial unifyheads product; chunked ReduceScatters sum the
partials across cores, each core returning row-shards of the final output.

Weight folding (host-side, exact algebra — the head dim equals emb here so
no information is lost):
    scores = q k^T = (y Wq)(x Wk)^T = y (Wq Wk^T) x^T
      ->  G  = Wk Wq^T   [e, e]   (host)
          g  = x G                 (device)
          scoresT[tk, tq] = g y^T  (device)
    out = attn @ v @ Wu = attn @ (x Wv Wu)
      ->  W2 = Wv Wu     [e, e]   (host)
          vW = x W2                (device)
This removes two of the four projection matmul groups.

Device compute in bf16 with fp32 PSUM accumulation, in "transposed"
layouts so no on-device transposes are needed (x/y transposed on host):
    gT = G^T @ xT            lhsT=G (natural),  rhs=xT       [e, t]
    vW = xT^T @ W2           lhsT=xT slices,    rhs=W2       [t, e]
    scoresT[tk,tq] = gT^T yT lhsT=gT slices,    rhs=yT
    expT = exp(scoresT * e^-0.5)  (softmax w/o max-subtraction; |scores|<~2)
    out[tq,:] = (expT^T @ vW) / (expT^T @ ones)  lhsT=expT slices (shared)
"""

import os
import numpy as np
import ml_dtypes

T, E, H = 2048, 512, 8
NCORES = 8
TB = 512          # matmul moving-operand block (free dim; one fp32 PSUM bank)
NE = E // 128     # 4  partition tiles of the emb dim
NT = T // 128     # 16 partition tiles of the seq dim
NB = T // TB      # 4  seq blocks
# ReduceScatter chunks (rows per chunk, sum = T).  The first two compute
# blocks share one chunk: with only two collectives ahead of the last one
# (both complete long before the final chunk's input is ready), the last —
# fully exposed — ReduceScatter can never queue up behind the CC chain on
# a skewed core.
CHUNKS = [1024, 512, 512]
NCH = len(CHUNKS)
CHUNK_R0 = [sum(CHUNKS[:c]) for c in range(NCH)]
EP = E + 1        # vW columns + folded ones column (softmax denominator)
NA = 257          # first-half free dim of the split final matmul
NBC = EP - NA     # 256; its last column holds the denominator

_cache = {}
last_result = None


def _build_nc():
    from concourse import bacc, tile
    from concourse.bass import mybir

    bf16 = mybir.dt.bfloat16
    f16 = mybir.dt.float16
    f32 = mybir.dt.float32

    nc = bacc.Bacc(
        "TRN2", target_bir_lowering=False, debug=False, num_devices=NCORES
    )

    xT = nc.dram_tensor("xT", [E, T], bf16, kind="ExternalInput")
    yT = nc.dram_tensor("yT", [E, T], bf16, kind="ExternalInput")
    gw = nc.dram_tensor("gw", [E, E], bf16, kind="ExternalInput")   # Wk Wq^T
    w2 = nc.dram_tensor("w2", [E, E], bf16, kind="ExternalInput")   # Wv Wu
    # fp16 reduction payload: the partials are ~N(0, 0.1^2) so fp16 keeps
    # ~3 more mantissa bits than bf16 and halves the collective bytes.
    out_exts = [
        nc.dram_tensor(f"out{c}", [CHUNKS[c] // NCORES, E], f16, kind="ExternalOutput")
        for c in range(NCH)
    ]

    with tile.TileContext(nc) as tc:
        with (
            tc.tile_pool(name="persist", bufs=1) as persist,
            tc.tile_pool(name="work", bufs=4) as work,
            tc.tile_pool(name="expp", bufs=32) as expp,
            tc.tile_pool(name="psum", bufs=2, space="PSUM") as psum_pool,
            tc.tile_pool(name="dram", bufs=1, space="DRAM") as dram,
        ):
            def alloc_rows(prefix, n):
                return [
                    persist.tile(
                        [128, n], bf16, tag=f"{prefix}{j}", name=f"{prefix}{j}"
                    )
                    for j in range(NE)
                ]

            xT_sb = alloc_rows("xTs", T)
            yT_sb = alloc_rows("yTs", T)
            gw_sb = alloc_rows("gws", E)
            w2_sb = alloc_rows("w2s", E)

            # DMA order = need order: gw, xT col-chunk 0, w2, rest of xT,
            # then yT per column block.  Column-chunked so the first
            # projection matmuls can start after ~1MB has landed.
            # first-needed tiles split in half so they spread across more
            # DMA queues and the first matmul can start sooner
            for j in range(NE):
                for h in range(2):
                    nc.sync.dma_start(
                        gw_sb[j][:, h * 256 : (h + 1) * 256],
                        gw[j * 128 : (j + 1) * 128, h * 256 : (h + 1) * 256],
                    )
                for h in range(2):
                    nc.sync.dma_start(
                        xT_sb[j][:, h * 256 : (h + 1) * 256],
                        xT[j * 128 : (j + 1) * 128, h * 256 : (h + 1) * 256],
                    )
            # w2 before the remaining xT blocks: the vW tiles for column
            # block 0 interleave with gT below, so w2 is needed early
            for j in range(NE):
                nc.sync.dma_start(w2_sb[j][:], w2[j * 128 : (j + 1) * 128, :])
            for tb in range(1, NB):
                for j in range(NE):
                    nc.sync.dma_start(
                        xT_sb[j][:, tb * TB : (tb + 1) * TB],
                        xT[j * 128 : (j + 1) * 128, tb * TB : (tb + 1) * TB],
                    )
            for tb in range(NB):
                for j in range(NE):
                    nc.sync.dma_start(
                        yT_sb[j][:, tb * TB : (tb + 1) * TB],
                        yT[j * 128 : (j + 1) * 128, tb * TB : (tb + 1) * TB],
                    )

            gT_sb = alloc_rows("gTs", T)
            # vW plus a folded ones column: col E is 1.0, so the final
            # matmul's second half also produces the softmax denominator.
            vW_sb = [
                persist.tile([128, EP], bf16, tag=f"vWs{t}", name=f"vWs{t}")
                for t in range(NT)
            ]

            zbias = persist.tile([128, 1], f32, tag="zbias", name="zbias")
            nc.vector.memset(zbias[:], 0.0)

            # Warm up the PE clock (HAM) during the initial DMA wait: dummy
            # matmuls on a zeroed tile keep TensorE busy so the ~3.4us
            # cold-clock ramp overlaps the input load instead of the first
            # real matmuls.
            if os.environ.get("KERNEL_WARMUP", "1") == "1":
                warm = persist.tile([128, TB], bf16, tag="warm", name="warm")
                nc.vector.memset(warm[:], 0.0)
                for w in range(8):
                    pw = psum_pool.tile(
                        [128, TB], f32, tag="mm", bufs=4, name="pw"
                    )
                    nc.tensor.matmul(
                        pw[:], warm[:, 0:128], warm[:], start=True, stop=True
                    )

            # gT[m][:, tk] = sum_j G[j][:, m-slice].T @ xT[j][:, tk-block]
            # vW[t, :]    = x @ W2   (natural [t, e] layout), ones in col E
            # Interleaved per xT column block: both only need block tb, so
            # each block gets ~6.6us of compute before the next must arrive —
            # double the DMA slack of a gT-then-vW ordering.
            for tb in range(NB):
                for m in range(NE):
                    ps = psum_pool.tile(
                        [128, TB], f32, tag="mm", bufs=4, name="ps_g"
                    )
                    for j in range(NE):
                        nc.tensor.matmul(
                            ps[:],
                            gw_sb[j][:, m * 128 : (m + 1) * 128],
                            xT_sb[j][:, tb * TB : (tb + 1) * TB],
                            start=(j == 0),
                            stop=(j == NE - 1),
                        )
                    nc.vector.tensor_copy(gT_sb[m][:, tb * TB : (tb + 1) * TB], ps[:])
                for t in range(4 * tb, 4 * tb + 4):
                    ps = psum_pool.tile([128, E], f32, tag="mm", bufs=4, name="ps_vw")
                    for j in range(NE):
                        nc.tensor.matmul(
                            ps[:],
                            xT_sb[j][:, t * 128 : (t + 1) * 128],
                            w2_sb[j][:],
                            start=(j == 0),
                            stop=(j == NE - 1),
                        )
                    nc.vector.memset(vW_sb[t][:, E:EP], 1.0)
                    nc.vector.tensor_copy(vW_sb[t][:, 0:E], ps[:])

            SCALE = float(E) ** -0.5
            parts = [
                dram.tile([CHUNKS[c], E], f16, tag=f"part{c}", name=f"part{c}")
                for c in range(NCH)
            ]
            rs_outs = [
                dram.tile(
                    [CHUNKS[c] // NCORES, E], f16, tag=f"rso{c}", name=f"rso{c}"
                )
                for c in range(NCH)
            ]
            # (chunk, row-tile within chunk) for each global 128-row tile
            tile2chunk = [
                (c, r) for c, nr in enumerate(CHUNKS) for r in range(nr // 128)
            ]

            for b in range(NB):
                # scoresT[tk, tq-block b] then exp
                exp_tiles = []
                for tk in range(NT):
                    ps = psum_pool.tile(
                        [128, TB], f32, tag="mm", bufs=4, name="ps_sc"
                    )
                    for m in range(NE):
                        nc.tensor.matmul(
                            ps[:],
                            gT_sb[m][:, tk * 128 : (tk + 1) * 128],
                            yT_sb[m][:, b * TB : (b + 1) * TB],
                            start=(m == 0),
                            stop=(m == NE - 1),
                        )
                    et = expp.tile([128, TB], bf16, tag="expT", bufs=32, name="et")
                    nc.scalar.activation(
                        et[:],
                        ps[:],
                        mybir.ActivationFunctionType.Exp,
                        bias=zbias[:],
                        scale=SCALE,
                    )
                    exp_tiles.append(et)

                # out rows for this block: accumulate over tk, then normalize.
                # The EP=513 free dim is split 257+256 across two PSUM banks;
                # the last column of pb is the softmax denominator.
                for qi in range(TB // 128):
                    pa = psum_pool.tile([128, NA], f32, tag="acca", bufs=2, name="pa")
                    pb = psum_pool.tile([128, NBC], f32, tag="accb", bufs=2, name="pb")
                    for tk in range(NT):
                        lhs = exp_tiles[tk][:, qi * 128 : (qi + 1) * 128]
                        nc.tensor.matmul(
                            pa[:],
                            lhs,
                            vW_sb[tk][:, 0:NA],
                            start=(tk == 0),
                            stop=(tk == NT - 1),
                        )
                        nc.tensor.matmul(
                            pb[:],
                            lhs,
                            vW_sb[tk][:, NA:EP],
                            start=(tk == 0),
                            stop=(tk == NT - 1),
                        )
                    rec = work.tile([128, 1], f32, tag="rec", bufs=4, name="rec")
                    nc.vector.reciprocal(rec[:], pb[:, NBC - 1 : NBC])
                    ot = work.tile([128, E], f16, tag="ot", bufs=4, name="ot")
                    nc.vector.tensor_scalar_mul(ot[:, 0:NA], pa[:], rec[:])
                    nc.scalar.mul(ot[:, NA:E], pb[:, 0 : NBC - 1], rec[:])
                    ch, r = tile2chunk[b * (TB // 128) + qi]
                    nc.sync.dma_start(
                        parts[ch][r * 128 : (r + 1) * 128, :], ot[:]
                    )
                    if r == CHUNKS[ch] // 128 - 1:
                        nc.gpsimd.collective_compute(
                            "ReduceScatter",
                            mybir.AluOpType.add,
                            replica_groups=[list(range(NCORES))],
                            ins=[parts[ch][:]],
                            outs=[rs_outs[ch][:]],
                        )
                        nc.sync.dma_start(out_exts[ch][:], rs_outs[ch][:])

    nc.compile()
    return nc


def kernel(x, y, Wk, Wq, Wv, Wu, bu):
    global last_result
    from concourse.bass_utils import run_bass_kernel_spmd

    if "nc" not in _cache:
        _cache["nc"] = _build_nc()
    nc = _cache["nc"]

    bf = ml_dtypes.bfloat16
    x = np.asarray(x, np.float32)
    y = np.asarray(y, np.float32)
    Wk = np.asarray(Wk, np.float32)
    Wq = np.asarray(Wq, np.float32)
    Wv = np.asarray(Wv, np.float32)
    Wu = np.asarray(Wu, np.float32)

    xT = np.ascontiguousarray(x.T).astype(bf)
    yT = np.ascontiguousarray(y.T).astype(bf)

    in_maps = []
    for i in range(NCORES):
        sl = slice(i * E, (i + 1) * E)
        G = Wk[:, sl] @ Wq[:, sl].T        # [e, e] fp32 on host
        W2 = Wv[:, sl] @ Wu[sl, :]         # [e, e] fp32 on host
        in_maps.append(
            {
                "xT": xT,
                "yT": yT,
                "gw": G.astype(bf),
                "w2": W2.astype(bf),
            }
        )

    trace = os.environ.get("KERNEL_TRACE", "0") == "1"
    res = run_bass_kernel_spmd(
        nc, in_maps, core_ids=list(range(NCORES)), trace=trace
    )
    last_result = res

    out_full = np.empty((T, E), np.float32)
    chunk_r0 = np.cumsum([0] + CHUNKS)[:-1]
    for i in range(NCORES):
        for c in range(NCH):
            nr = CHUNKS[c] // NCORES
            o = np.asarray(res.results[i][f"out{c}"]).astype(np.float32)
            r0 = chunk_r0[c] + i * nr
            out_full[r0 : r0 + nr] = o
    out_full = out_full + np.asarray(bu, np.float32)[None, :]
    return out_full[None]
